# revision 1
# baseline (speedup 1.0000x reference)
"""Trainium2 kernel for the OpticalFront dense net.

Reference computation:
    xr = Re(idft2(tmask * dft2(x)))          # centered 2D FFT front
    h = relu(xr.flat @ w1.T + b1)
    out = log_softmax(h @ w4.T + b4)

The optical front is a fixed real-linear operator A on each flattened
28x28 image (xr_flat = x_flat @ A.T), so it folds into the first FC
layer on the host: w1_eff = w1 @ A.  The device then runs a pure GEMM
pipeline, data-parallel over 8 NeuronCores (4096 images per core).

FC1 runs entirely in fp8-e4m3 DoubleRow mode (2 contraction rows per
PE cell, ~2x the bf16 matmul rate): pixels 0..767 in 3 DoubleRow
matmuls of 256 virtual rows, pixels 768..783 plus the bias row in one
thin 9-partition DoubleRow matmul, all accumulating into one PSUM
bank.  Keeping every FC1 matmul in the same fp8 mode avoids the PE
drain/refill that mixed-dtype back-to-back matmuls cost.  w1 is
pre-scaled by 64 so its fp8 encoding sits in e4m3's normal range; the
1/64 is folded into w4 on the host.  FC2 and the log-softmax stay
bf16/fp32 (fp8 there would blow the error budget).

    H'.T[hid, b] = sum_t W18[t].T @ X8[t]                   (fp32 acc)
    L.T[10,  b] = sum_k W4T[k, 10].T  @ H'.T[k, b]          (w4/64)
    out.T[10, b] = (L.T + b4) - ln(ones.T @ exp(L.T + b4))

Layout: contraction dims on the SBUF partition axis; batch streams
along the free axis in chunks of 512 (one PSUM bank).
"""

import numpy as np
import ml_dtypes

import concourse.bass as bass
import concourse.bacc as bacc
import concourse.mybir as mybir
import concourse.tile as tile
from concourse.bass_utils import run_bass_kernel_spmd

BF16 = mybir.dt.bfloat16
FP8 = mybir.dt.float8e4
F32 = mybir.dt.float32
AF = mybir.ActivationFunctionType
DR = mybir.MatmulPerfMode.DoubleRow

B, H, W = 32768, 28, 28
PIX = H * W            # 784
HID = 800
NCLS = 10
NCORES = 8
BPC = B // NCORES      # 4096 images per core
NB = 512               # batch chunk = one PSUM bank of fp32
NCH = BPC // NB        # 8 chunks per core
NT8 = 3                # full fp8 DoubleRow tiles (256 pixels each = 768)
P8 = NT8 * 256         # pixels covered by the full fp8 tiles
KT4 = 9                # thin tile partitions: 18 virtual rows >= 16 px + bias
WS = 64.0              # host-side scale on w1/b1 (folded out via w4)
HT = (HID + 127) // 128          # 7 contraction tiles for fc2
M_TILES = [(m * 128, min(128, HID - m * 128)) for m in range(HT)]
NBD = 1024             # x DMA chunk width
NDCH = BPC // NBD      # 4 DMA chunks per core

_built = None  # nc cache — BIR build is pure host work


def _build_device_program():
    nc = bacc.Bacc(
        "TRN2", target_bir_lowering=False, debug=False, num_devices=NCORES
    )
    # x8 packed per DMA chunk: per partition p, chunk dc holds the
    # contiguous 6KB block [t, i, n] with pixel k = 256 t + 2 p + i.
    x8_d = nc.dram_tensor("x8", [128, NDCH, NT8, 2, NBD], FP8, kind="ExternalInput")
    # thin tail tile: virtual row v = 2 p + i -> pixel 768+v (v<16),
    # v==16 the ones/bias row, v==17 zero pad.
    x4_d = nc.dram_tensor("x4", [KT4, NDCH, 2, NBD], FP8, kind="ExternalInput")
    w18_d = nc.dram_tensor("w18", [128, NT8, 2, HID], FP8, kind="ExternalInput")
    w4_8d = nc.dram_tensor("wt4", [KT4, 2, HID], FP8, kind="ExternalInput")
    w4t_d = nc.dram_tensor("w4t", [HT * 128, NCLS], BF16, kind="ExternalInput")
    b4_d = nc.dram_tensor("b4", [NCLS, 1], F32, kind="ExternalInput")
    ones_d = nc.dram_tensor("ones", [NCLS, NCLS], BF16, kind="ExternalInput")
    out_d = nc.dram_tensor("outT", [NCLS, BPC], F32, kind="ExternalOutput")

    # The one ACT-function table containing relu/exp/ln/identity (avoids
    # per-transition LUT reloads).  Loaded AFTER the first DMAs are issued
    # on the scalar queue so it doesn't delay the first matmul's weights.
    from concourse.hw_specs import get_activation_tables
    needed = {AF.Relu, AF.Exp, AF.Ln, AF.Identity, AF.Copy}
    table_id = None
    for i, (name, funcs) in enumerate(get_activation_tables(nc.m.arch).items()):
        if needed <= funcs:
            table_id = i
            break

    with tile.TileContext(nc) as tc:
        with (
            tc.tile_pool(name="weights", bufs=1) as wpool,
            tc.tile_pool(name="xin", bufs=3) as xpool,
            tc.tile_pool(name="hmid", bufs=2 * HT) as hpool,
            tc.tile_pool(name="smax", bufs=4) as spool,
            tc.tile_pool(name="psum_h", bufs=3, space="PSUM") as psum_h,
            tc.tile_pool(name="psum_l", bufs=2, space="PSUM") as psum_l_pool,
            tc.tile_pool(name="psum_t", bufs=1, space="PSUM") as psum_t,
        ):
            w4_view = w4t_d.ap().rearrange("(k p) m -> p k m", p=128)

            # Startup order: the first m-group needs w18 t0 (scalar
            # queue) and the first half-chunk of x8 (sync queue); issue
            # those first on their queues, everything else after.
            w18_sb = wpool.tile([128, NT8, 2, HID], FP8)
            nc.scalar.dma_start(w18_sb[:, 0:1, :, :], w18_d[:, 0:1, :, :])

            xts = {}   # dma chunk -> [128, NT8, 2, NBD] tile
            x4s = {}   # dma chunk -> [KT4, 2, NBD] tile

            def load_chunk(dc, split=False):
                x8_sb = xpool.tile([128, NT8, 2, NBD], FP8, tag="x8")
                x4_sb = xpool.tile([KT4, 2, NBD], FP8, tag="x4")
                if split:
                    nc.sync.dma_start(
                        x8_sb[:, :, :, 0:NB], x8_d[:, dc, :, :, 0:NB]
                    )
                    nc.sync.dma_start(x4_sb[:, :, :], x4_d[:, dc, :, :])
                    nc.sync.dma_start(
                        x8_sb[:, :, :, NB:NBD], x8_d[:, dc, :, :, NB:NBD]
                    )
                else:
                    nc.sync.dma_start(x8_sb[:, :, :, :], x8_d[:, dc, :, :, :])
                    nc.sync.dma_start(x4_sb[:, :, :], x4_d[:, dc, :, :])
                xts[dc] = x8_sb
                x4s[dc] = x4_sb

            load_chunk(0, split=True)

            nc.scalar.dma_start(w18_sb[:, 1:NT8, :, :], w18_d[:, 1:NT8, :, :])
            wt4_sb = wpool.tile([KT4, 2, HID], FP8)
            nc.scalar.dma_start(wt4_sb[:, :, :], w4_8d[:, :, :])

            if table_id is not None:
                nc.scalar.add_instruction(
                    mybir.InstLoadActFuncSet(
                        name=nc.get_next_instruction_name(),
                        act_func_set_id=table_id,
                        ins=[],
                        outs=[],
                    )
                )

            w4_sb = wpool.tile([128, HT, NCLS], BF16)
            nc.gpsimd.dma_start(w4_sb[:, :, :], w4_view)
            b4_sb = wpool.tile([NCLS, 1], F32)
            nc.gpsimd.dma_start(b4_sb[:, :], b4_d[:, :])
            ones_sb = wpool.tile([NCLS, NCLS], BF16)
            nc.gpsimd.dma_start(ones_sb[:, :], ones_d[:, :])

            # Software pipeline: nb's softmax PE ops (partition-sum and
            # broadcast matmuls) are emitted inside nb+1's fc1 m-loop so
            # the PE never stalls waiting on ScalarE's exp/ln.
            stage1 = None  # after exp: ones-matmul partition sum + ln
            stage2 = None  # after ln: broadcast matmul + bias/sub + store

            def softmax_stage1():
                nonlocal stage1, stage2
                if stage1 is None:
                    return
                pl, exp_sb, nb = stage1
                stage1 = None
                ps = psum_t.tile([1, NB], F32, tag="ps")
                nc.tensor.matmul(ps[:, :], ones_sb[:, 0:1], exp_sb[:, :])
                lse_sb = spool.tile([1, NB], BF16, tag="lse")
                nc.scalar.activation(lse_sb[:, :], ps[:, :], AF.Ln)
                stage2 = (pl, lse_sb, nb)

            def softmax_stage2():
                nonlocal stage2
                if stage2 is None:
                    return
                pl, lse_sb, nb = stage2
                stage2 = None
                pb = psum_t.tile([NCLS, NB], F32, tag="pb")
                nc.tensor.matmul(pb[:, :], ones_sb[0:1, :], lse_sb[:, :])
                logit_sb = spool.tile([NCLS, NB], F32, tag="logit")
                nc.vector.tensor_scalar_add(logit_sb[:, :], pl[:, :], b4_sb[:, :])
                out_sb = spool.tile([NCLS, NB], F32, tag="outc")
                nc.vector.tensor_sub(out_sb[:, :], logit_sb[:, :], pb[:, :])
                nc.scalar.dma_start(out_d[:, nb * NB:(nb + 1) * NB], out_sb[:, :])

            for nb in range(NCH):
                dc, half = divmod(nb, NBD // NB)
                if half == 0 and dc + 1 < NDCH:
                    load_chunk(dc + 1)
                hs = slice(half * NB, (half + 1) * NB)

                hts = []
                for m, (m0, mm) in enumerate(M_TILES):
                    ph = psum_h.tile([128, NB], F32, tag="ph")
                    for t in range(NT8):
                        nc.tensor.matmul(
                            ph[:mm, :],
                            w18_sb[:, t, :, m0:m0 + mm],
                            xts[dc][:, t, :, hs],
                            start=(t == 0),
                            stop=False,
                            perf_mode=DR,
                        )
                    nc.tensor.matmul(
                        ph[:mm, :],
                        wt4_sb[:, :, m0:m0 + mm],
                        x4s[dc][:, :, hs],
                        start=False,
                        stop=True,
                        perf_mode=DR,
                    )
                    ht = hpool.tile([128, NB], BF16, tag="ht")
                    nc.scalar.activation(ht[:mm, :], ph[:mm, :], AF.Relu)
                    hts.append(ht)
                    if m == 1:
                        softmax_stage1()  # previous nb: sum-of-exp matmul
                    elif m == 3:
                        softmax_stage2()  # previous nb: broadcast + store

                pl = psum_l_pool.tile([NCLS, NB], F32, tag="pl")
                for k in range(HT):
                    kk = min(128, HID - k * 128)
                    nc.tensor.matmul(
                        pl[:, :],
                        w4_sb[:kk, k, :],
                        hts[k][:kk, :],
                        start=(k == 0),
                        stop=(k == HT - 1),
                    )
                exp_sb = spool.tile([NCLS, NB], BF16, tag="exp")
                nc.scalar.activation(exp_sb[:, :], pl[:, :], AF.Exp, bias=b4_sb[:, :])
                stage1 = (pl, exp_sb, nb)
            softmax_stage1()
            softmax_stage2()

    nc.finalize()
    return nc


def _optical_operator(tmask_re, tmask_im):
    """A such that xr_flat = A @ x_flat for the masked centered FFT front."""
    tmask = tmask_re.astype(np.complex64) + 1j * tmask_im.astype(np.complex64)
    tmask = tmask.reshape(H, W)
    ax = (-2, -1)
    eye = np.eye(PIX, dtype=np.complex64).reshape(PIX, H, W)
    f = np.fft.fftshift(np.fft.fft2(np.fft.ifftshift(eye, axes=ax), axes=ax), axes=ax)
    f *= tmask[None, :, :]
    xr = np.fft.fftshift(np.fft.ifft2(np.fft.ifftshift(f, axes=ax), axes=ax), axes=ax)
    return np.real(xr).reshape(PIX, PIX).T.astype(np.float64)


def kernel(x, tmask_re, tmask_im, w1, b1, w4, b4):
    global _built
    x = np.asarray(x)
    w1 = np.asarray(w1, dtype=np.float32)
    b1 = np.asarray(b1, dtype=np.float32)
    w4 = np.asarray(w4, dtype=np.float32)
    b4 = np.asarray(b4, dtype=np.float32)
    tre = np.asarray(tmask_re, dtype=np.float32)
    tim = np.asarray(tmask_im, dtype=np.float32)

    # Fold the optical front into w1.  Identity mask -> A == I exactly.
    if np.all(tre == 1.0) and np.all(tim == 0.0):
        w1e = w1.astype(np.float64)
    else:
        w1e = w1.astype(np.float64) @ _optical_operator(tre, tim)

    bf16 = ml_dtypes.bfloat16
    fp8 = ml_dtypes.float8_e4m3fn

    def q8(a):
        return np.clip(a, -240, 240).astype(fp8)

    # w1 scaled by WS so fp8 encodings sit in e4m3's normal range; the
    # matching 1/WS rides on w4 (relu commutes with positive scaling).
    w1s = (w1e * WS).astype(np.float32)
    # full fp8 tiles: [hid, 768] -> [128 p, 3 t, 2 i, hid], k = 256t+2p+i
    w18 = np.ascontiguousarray(
        q8(w1s[:, :P8]).reshape(HID, NT8, 128, 2).transpose(2, 1, 3, 0)
    )
    # thin tile: rows v = 2p+i -> pixel 768+v (v<16), v=16 bias, v=17 pad
    wt4f = np.zeros((2 * KT4, HID), dtype=np.float32)
    wt4f[:PIX - P8, :] = w1s[:, P8:].T
    wt4f[PIX - P8, :] = (b1 * WS).astype(np.float32)
    wt4 = np.ascontiguousarray(q8(wt4f).reshape(KT4, 2, HID))

    w4t = np.zeros((HT * 128, NCLS), dtype=bf16)
    w4t[:HID, :] = (w4 / WS).T
    b4c = np.ascontiguousarray(b4.reshape(NCLS, 1))
    ones = np.ones((NCLS, NCLS), dtype=bf16)

    # x: [B, 784] fp8, packed to match the weight tiles, plus per-core
    # DMA chunking: [128 p, NDCH dc, 3 t, 2 i, NBD n] and the thin tile
    # [9 p, NDCH, 2 i, NBD] whose row 16 is the constant ones row.
    xf = x.reshape(B, PIX)
    x8 = q8(xf[:, :P8]).reshape(B, NT8, 128, 2).transpose(2, 0, 1, 3)
    x4f = np.zeros((2 * KT4, B), dtype=np.float32)
    x4f[:PIX - P8, :] = xf[:, P8:].T
    x4f[PIX - P8, :] = 1.0
    x4 = q8(x4f).reshape(KT4, 2, B)

    if _built is None:
        _built = _build_device_program()
    nc = _built

    in_maps = []
    for c in range(NCORES):
        sl = slice(c * BPC, (c + 1) * BPC)
        # [128, BPC, 3, 2] -> [128, NDCH, NBD, 3, 2] -> [128, NDCH, 3, 2, NBD]
        x8c = x8[:, sl].reshape(128, NDCH, NBD, NT8, 2).transpose(0, 1, 3, 4, 2)
        # [9, 2, BPC] -> [9, 2, NDCH, NBD] -> [9, NDCH, 2, NBD]
        x4c = x4[:, :, sl].reshape(KT4, 2, NDCH, NBD).transpose(0, 2, 1, 3)
        in_maps.append({
            "x8": np.ascontiguousarray(x8c),
            "x4": np.ascontiguousarray(x4c),
            "w18": w18,
            "wt4": wt4,
            "w4t": w4t,
            "b4": b4c,
            "ones": ones,
        })
    res = run_bass_kernel_spmd(nc, in_maps, core_ids=list(range(NCORES)))

    out = np.empty((B, NCLS), dtype=np.float32)
    for c in range(NCORES):
        out[c * BPC:(c + 1) * BPC, :] = res.results[c]["outT"].T
    return out



# revision 4
# speedup vs baseline: 1.1669x; 1.1669x over previous
"""Trainium2 kernel for the OpticalFront dense net.

Reference computation:
    xr = Re(idft2(tmask * dft2(x)))          # centered 2D FFT front
    h = relu(xr.flat @ w1.T + b1)
    out = log_softmax(h @ w4.T + b4)

The optical front is a fixed real-linear operator A on each flattened
28x28 image (xr_flat = x_flat @ A.T), so it folds into the first FC
layer on the host: w1_eff = w1 @ A.  The device then runs a pure GEMM
pipeline, data-parallel over 8 NeuronCores (4096 images per core).

Device structure (per core, 8 batch chunks of 512, grouped 4+4):

  FC1 runs in fp8-e4m3 DoubleRow mode, m-tile-major over each group of
  4 chunks so each weight tile is loaded into the PE array once and
  streamed against 4 chunks.  A post-build IR pass strips the redundant
  LDWEIGHTS that bass emits per matmul (the hardware keeps the
  stationary operand between matmuls), which removes the dominant
  ~130ns/matmul weight-reload tax of a chunk-major loop.  b1 rides on
  the relu activation's per-partition bias port (fp32, not fp8).

  FC2 packs the four chunks' [10, 512] logit tiles into one PSUM bank
  at partition strips {0,32,64,96} via tile_position column groups;
  consecutive strip matmuls stream different chunks through different
  XBUSes and overlap in the array.  The log-softmax partition
  reductions use the ones-matmul trick on diagonal (row,col)=(32c,32c)
  tiles, and exp/ln/bias-add/sub each run as ONE wide instruction over
  all strips instead of per-chunk.  Relus alternate between ScalarE
  and VectorE so neither engine sits on the critical path.

    H'.T[hid, b] = sum_t W18[t].T @ X8[t]                   (fp32 acc)
    L.T[strip c][10, b] = sum_k (W4/64)[k].T @ H'.T[k, b]
    out.T = (L.T + b4) - ln(ones.T @ exp(L.T + b4))
"""

import numpy as np
import ml_dtypes

import concourse.bass as bass
import concourse.bacc as bacc
import concourse.mybir as mybir
import concourse.tile as tile
from concourse.bass_utils import run_bass_kernel_spmd

BF16 = mybir.dt.bfloat16
FP8 = mybir.dt.float8e4
F32 = mybir.dt.float32
AF = mybir.ActivationFunctionType
DR = mybir.MatmulPerfMode.DoubleRow
ALU = mybir.AluOpType

B, H, W = 32768, 28, 28
PIX = H * W            # 784
HID = 800
NCLS = 10
NCORES = 8
BPC = B // NCORES      # 4096 images per core
NB = 512               # batch chunk = one PSUM bank of fp32
NCH = BPC // NB        # 8 chunks per core
NT8 = 3                # full fp8 DoubleRow tiles (256 pixels each = 768)
P8 = NT8 * 256         # pixels covered by the full fp8 tiles
KT3 = 8                # tail tile partitions: 16 virtual rows = 16 px
WS = 64.0              # host-side scale on w1/b1 (folded out via w4)
HT = (HID + 127) // 128          # 7 contraction tiles for fc2
M_TILES = [(m * 128, min(128, HID - m * 128)) for m in range(HT)]
GROUPS = [(0, 4), (4, 4)]        # (first chunk, chunks per group)

_built = None  # nc cache — BIR build is pure host work


def _dedup_ldweights(nc):
    """Remove Ldweights whose weight AP/mode matches the immediately
    preceding Ldweights on the tensor queue: the PE array keeps its
    stationary operand between matmuls, so repeated loads are pure
    overhead.  Any waits/updates on a removed load migrate to the next
    tensor-engine instruction (its matmul)."""
    removed = 0
    for blk in nc.main_func.blocks:
        last_sig = None
        to_remove = []
        for ins in blk.instructions:
            op = ins.opcode
            if op == 'Ldweights':
                sig = (str(ins.ins[0]), str(ins.perf_mode),
                       str(ins.tile_position), str(ins.is_transpose))
                if sig == last_sig:
                    to_remove.append(ins)
                    continue
                last_sig = sig
        for ins in to_remove:
            arr = blk.instructions
            idx = next(i for i in range(len(arr)) if arr[i].name == ins.name)
            # next tensor-engine instruction inherits the sync info
            nxt = None
            for j in range(idx + 1, len(arr)):
                if arr[j].engine == ins.engine:
                    nxt = arr[j]
                    break
            assert nxt is not None
            si = ins.sync_info
            if si is not None and (len(si.on_wait) or len(si.on_update)):
                nsi = nxt.sync_info
                if nsi is None:
                    nxt.sync_info = mybir.SyncInfo(
                        on_wait=list(si.on_wait), on_update=list(si.on_update))
                else:
                    nsi.on_wait = list(nsi.on_wait) + list(si.on_wait)
                    nsi.on_update = list(nsi.on_update) + list(si.on_update)
            del arr[idx]
            removed += 1
    return removed


def _build_device_program():
    nc = bacc.Bacc(
        "TRN2", target_bir_lowering=False, debug=False, num_devices=NCORES
    )
    # x8: pixel k = 256 t + 2 p + i, image n = 512 c + j.
    x8_d = nc.dram_tensor("x8", [NT8, 128, NCH, 2, NB], FP8, kind="ExternalInput")
    # tail: virtual row v = 2 p + i -> pixel 768 + v.
    x3_d = nc.dram_tensor("x3", [KT3, NCH, 2, NB], FP8, kind="ExternalInput")
    w18_d = nc.dram_tensor("w18", [128, NT8, 2, HID], FP8, kind="ExternalInput")
    wt3_d = nc.dram_tensor("wt3", [KT3, 2, HID], FP8, kind="ExternalInput")
    w4_d = nc.dram_tensor("w4", [128, HT, NCLS], BF16, kind="ExternalInput")
    b1_d = nc.dram_tensor("b1", [128, HT], F32, kind="ExternalInput")
    b4r_d = nc.dram_tensor("b4r", [128, 1], F32, kind="ExternalInput")
    out_d = nc.dram_tensor("outT", [NCLS, BPC], F32, kind="ExternalOutput")

    # One ACT-function table containing relu/exp/ln (avoids LUT reloads).
    from concourse.hw_specs import get_activation_tables
    needed = {AF.Relu, AF.Exp, AF.Ln, AF.Identity, AF.Copy}
    table_id = None
    for i, (name, funcs) in enumerate(get_activation_tables(nc.m.arch).items()):
        if needed <= funcs:
            table_id = i
            break

    with tile.TileContext(nc) as tc:
        with (
            tc.tile_pool(name="weights", bufs=1) as wpool,
            tc.tile_pool(name="xin", bufs=2) as xpool,
            tc.tile_pool(name="hmid", bufs=40) as hpool,
            tc.tile_pool(name="smax", bufs=2) as spool,
            tc.tile_pool(name="psum_h", bufs=6, space="PSUM") as php,
            tc.tile_pool(name="psum_l", bufs=1, space="PSUM") as plp,
            tc.tile_pool(name="psum_t", bufs=1, space="PSUM") as smp,
        ):
            # ---- startup DMAs: first-needed first, on separate queues ----
            w18_sb = wpool.tile([128, NT8, 2, HID], FP8)
            nc.scalar.dma_start(w18_sb[:, 0:1, :, :], w18_d[:, 0:1, :, :])

            xg = {}   # (group, t) -> [128, G, 2, NB]
            x3g = {}  # group -> [KT3, G, 2, NB]

            def load_group(gi):
                c0, G = GROUPS[gi]
                for t in range(NT8):
                    xt = xpool.tile([128, G, 2, NB], FP8, tag=f"x{t}")
                    nc.sync.dma_start(xt[:, :, :, :], x8_d[t, :, c0:c0 + G, :, :])
                    xg[(gi, t)] = xt
                x3t = xpool.tile([KT3, G, 2, NB], FP8, tag="x3")
                nc.sync.dma_start(x3t[:, :, :, :], x3_d[:, c0:c0 + G, :, :])
                x3g[gi] = x3t

            load_group(0)

            nc.scalar.dma_start(w18_sb[:, 1:NT8, :, :], w18_d[:, 1:NT8, :, :])
            wt3_sb = wpool.tile([KT3, 2, HID], FP8)
            nc.scalar.dma_start(wt3_sb[:, :, :], wt3_d[:, :, :])

            if table_id is not None:
                nc.scalar.add_instruction(
                    mybir.InstLoadActFuncSet(
                        name=nc.get_next_instruction_name(),
                        act_func_set_id=table_id,
                        ins=[],
                        outs=[],
                    )
                )

            w4_sb = wpool.tile([128, HT, NCLS], BF16)
            nc.gpsimd.dma_start(w4_sb[:, :, :], w4_d[:, :, :])
            b1_sb = wpool.tile([128, HT], F32)
            nc.gpsimd.dma_start(b1_sb[:, :], b1_d[:, :])
            b4r_sb = wpool.tile([128, 1], F32)
            nc.gpsimd.dma_start(b4r_sb[:, :], b4r_d[:, :])
            ones_sb = wpool.tile([128, 16], BF16)
            nc.gpsimd.memset(ones_sb[:, :], 1.0)

            load_group(1)

            # Software-pipelined softmax: group g's partition reductions
            # (PE) are emitted inside group g+1's FC1 m-loop so the PE
            # never stalls on ScalarE's exp/ln.
            stage1 = None  # (exp_sb, logit_sb, c0, G)
            stage2 = None  # (lse_sb, logit_sb, c0, G)

            def softmax_stage1():
                nonlocal stage1, stage2
                if stage1 is None:
                    return
                exp_sb, logit_sb, c0, G = stage1
                stage1 = None
                ps_t = smp.tile([128, NB], F32, tag="sm")
                for ci in range(G):
                    s = 32 * ci
                    nc.tensor.matmul(
                        ps_t[s:s + 1, :],
                        ones_sb[s:s + NCLS, 0:1],
                        exp_sb[s:s + NCLS, :],
                        tile_position=(s, s),
                    )
                lse_sb = spool.tile([128, NB], BF16, tag="lse")
                top1 = 32 * (G - 1) + 1
                nc.scalar.activation(lse_sb[:top1, :], ps_t[:top1, :], AF.Ln)
                stage2 = (lse_sb, logit_sb, c0, G)

            def softmax_stage2():
                nonlocal stage2
                if stage2 is None:
                    return
                lse_sb, logit_sb, c0, G = stage2
                stage2 = None
                top = 32 * (G - 1) + NCLS
                pb_t = smp.tile([128, NB], F32, tag="sm")
                for ci in range(G):
                    s = 32 * ci
                    nc.tensor.matmul(
                        pb_t[s:s + NCLS, :],
                        ones_sb[s:s + 1, 0:NCLS],
                        lse_sb[s:s + 1, :],
                        tile_position=(s, s),
                    )
                out_sb = spool.tile([128, NB], F32, tag="outc")
                nc.vector.tensor_sub(out_sb[:top, :], logit_sb[:top, :], pb_t[:top, :])
                for ci in range(G):
                    s = 32 * ci
                    c = c0 + ci
                    nc.gpsimd.dma_start(
                        out_d[:, c * NB:(c + 1) * NB], out_sb[s:s + NCLS, :]
                    )

            for gi, (c0, G) in enumerate(GROUPS):
                top = 32 * (G - 1) + NCLS
                hts = {}  # (ci, m) -> relu'd fc1 tile
                pl_t = None

                def fc2_block(k):
                    kk = min(128, HID - k * 128)
                    for ci in range(G):
                        s = 32 * ci
                        nc.tensor.matmul(
                            pl_t[s:s + NCLS, :],
                            w4_sb[:kk, k, :],
                            hts[(ci, k)][:kk, :],
                            start=(k == 0),
                            stop=(k == HT - 1),
                            tile_position=(0, s),
                        )

                for m, (m0, mm) in enumerate(M_TILES):
                    ph_t = [
                        php.tile([128, NB], F32, tag="ph", name=f"ph{gi}_{m}_{ci}")
                        for ci in range(G)
                    ]
                    for t in range(NT8 + 1):
                        for ci in range(G):
                            if t < NT8:
                                lhsT = w18_sb[:, t, :, m0:m0 + mm]
                                rhs = xg[(gi, t)][:, ci, :, :]
                            else:
                                lhsT = wt3_sb[:, :, m0:m0 + mm]
                                rhs = x3g[gi][:, ci, :, :]
                            nc.tensor.matmul(
                                ph_t[ci][:mm, :],
                                lhsT,
                                rhs,
                                start=(t == 0),
                                stop=(t == NT8),
                                perf_mode=DR,
                            )
                    for ci in range(G):
                        ht_t = hpool.tile([128, NB], BF16, tag="ht")
                        if (m + ci) % 2 == 0:
                            nc.scalar.activation(
                                ht_t[:mm, :], ph_t[ci][:mm, :], AF.Relu,
                                bias=b1_sb[:mm, m:m + 1],
                            )
                        else:
                            nc.vector.tensor_scalar(
                                ht_t[:mm, :], ph_t[ci][:mm, :],
                                b1_sb[:mm, m:m + 1], 0.0, ALU.add, ALU.max,
                            )
                        hts[(ci, m)] = ht_t
                    if m == 0:
                        pl_t = plp.tile([128, NB], F32, tag="pl")
                    if m >= 1:
                        fc2_block(m - 1)  # this group's fc2, k = m-1
                    if m == 1:
                        softmax_stage1()  # previous group
                    elif m == 3:
                        softmax_stage2()  # previous group

                fc2_block(HT - 1)
                exp_sb = spool.tile([128, NB], BF16, tag="exp")
                nc.scalar.activation(
                    exp_sb[:top, :], pl_t[:top, :], AF.Exp, bias=b4r_sb[:top, 0:1]
                )
                logit_sb = spool.tile([128, NB], F32, tag="logit")
                nc.vector.tensor_scalar_add(
                    logit_sb[:top, :], pl_t[:top, :], b4r_sb[:top, 0:1]
                )
                stage1 = (exp_sb, logit_sb, c0, G)

            softmax_stage1()
            softmax_stage2()

    n = _dedup_ldweights(nc)
    assert n > 0, "ldweights dedup pass matched nothing"
    nc.finalize()
    return nc


def _optical_operator(tmask_re, tmask_im):
    """A such that xr_flat = A @ x_flat for the masked centered FFT front."""
    tmask = tmask_re.astype(np.complex64) + 1j * tmask_im.astype(np.complex64)
    tmask = tmask.reshape(H, W)
    ax = (-2, -1)
    eye = np.eye(PIX, dtype=np.complex64).reshape(PIX, H, W)
    f = np.fft.fftshift(np.fft.fft2(np.fft.ifftshift(eye, axes=ax), axes=ax), axes=ax)
    f *= tmask[None, :, :]
    xr = np.fft.fftshift(np.fft.ifft2(np.fft.ifftshift(f, axes=ax), axes=ax), axes=ax)
    return np.real(xr).reshape(PIX, PIX).T.astype(np.float64)


def kernel(x, tmask_re, tmask_im, w1, b1, w4, b4):
    global _built
    x = np.asarray(x)
    w1 = np.asarray(w1, dtype=np.float32)
    b1 = np.asarray(b1, dtype=np.float32)
    w4 = np.asarray(w4, dtype=np.float32)
    b4 = np.asarray(b4, dtype=np.float32)
    tre = np.asarray(tmask_re, dtype=np.float32)
    tim = np.asarray(tmask_im, dtype=np.float32)

    # Fold the optical front into w1.  Identity mask -> A == I exactly.
    if np.all(tre == 1.0) and np.all(tim == 0.0):
        w1e = w1.astype(np.float64)
    else:
        w1e = w1.astype(np.float64) @ _optical_operator(tre, tim)

    bf16 = ml_dtypes.bfloat16
    fp8 = ml_dtypes.float8_e4m3fn

    def q8(a):
        return np.clip(a, -240, 240).astype(fp8)

    # w1 scaled by WS so fp8 encodings sit in e4m3's normal range; the
    # matching 1/WS rides on w4 (relu commutes with positive scaling).
    w1s = (w1e * WS).astype(np.float32)
    # full fp8 tiles: [hid, 768] -> [128 p, 3 t, 2 i, hid], k = 256t+2p+i
    w18 = np.ascontiguousarray(
        q8(w1s[:, :P8]).reshape(HID, NT8, 128, 2).transpose(2, 1, 3, 0)
    )
    # tail tile: rows v = 2p+i -> pixel 768+v (v < 16)
    wt3 = np.ascontiguousarray(
        q8(w1s[:, P8:]).reshape(HID, KT3, 2).transpose(1, 2, 0)
    )
    w4t = np.zeros((HT * 128, NCLS), dtype=np.float32)
    w4t[:HID, :] = (w4 / WS).T
    w4c = np.ascontiguousarray(
        w4t.reshape(HT, 128, NCLS).transpose(1, 0, 2).astype(bf16)
    )
    b1f = np.zeros(HT * 128, dtype=np.float32)
    b1f[:HID] = b1 * WS
    b1c = np.ascontiguousarray(b1f.reshape(HT, 128).T)
    b4r = np.zeros((128, 1), dtype=np.float32)
    for c in range(4):
        b4r[32 * c:32 * c + NCLS, 0] = b4

    # x: [B, 784] fp8 packed as [3 t, 128 p, 8 c, 2 i, 512 j] per core,
    # plus the tail tile [8 p, 8 c, 2 i, 512 j].
    xf = x.reshape(B, PIX)
    x8a = q8(xf[:, :P8])       # [B, 768]
    x3a = q8(xf[:, P8:])       # [B, 16]

    if _built is None:
        _built = _build_device_program()
    nc = _built

    in_maps = []
    for c in range(NCORES):
        sl = slice(c * BPC, (c + 1) * BPC)
        # [BPC, 768] -> [8, 512, 3, 128, 2] -> [3, 128, 8, 2, 512]
        x8c = np.ascontiguousarray(
            x8a[sl].reshape(NCH, NB, NT8, 128, 2).transpose(2, 3, 0, 4, 1)
        )
        # [BPC, 16] -> [8, 512, 8, 2] -> [8, 8, 2, 512]
        x3c = np.ascontiguousarray(
            x3a[sl].reshape(NCH, NB, KT3, 2).transpose(2, 0, 3, 1)
        )
        in_maps.append({
            "x8": x8c,
            "x3": x3c,
            "w18": w18,
            "wt3": wt3,
            "w4": w4c,
            "b1": b1c,
            "b4r": b4r,
        })
    res = run_bass_kernel_spmd(nc, in_maps, core_ids=list(range(NCORES)))

    out = np.empty((B, NCLS), dtype=np.float32)
    for c in range(NCORES):
        out[c * BPC:(c + 1) * BPC, :] = res.results[c]["outT"].T
    return out


# revision 14
# speedup vs baseline: 1.4049x; 1.2039x over previous
"""Trainium2 kernel for the OpticalFront dense net.

Reference computation:
    xr = Re(idft2(tmask * dft2(x)))          # centered 2D FFT front
    h = relu(xr.flat @ w1.T + b1)
    out = log_softmax(h @ w4.T + b4)

The optical front is a fixed real-linear operator A on each flattened
28x28 image (xr_flat = x_flat @ A.T), so it folds into the first FC
layer on the host: w1_eff = w1 @ A.  The device then runs a pure GEMM
pipeline, data-parallel over 8 NeuronCores (4096 images per core).

Device structure (per core, 8 batch chunks of 512, grouped 4+4):

  FC1 runs in fp8-e4m3 DoubleRow mode, m-tile-major over each group of
  4 chunks so each weight tile is loaded into the PE array once and
  streamed against 4 chunks.  A post-build IR pass strips the redundant
  LDWEIGHTS that bass emits per matmul (the hardware keeps the
  stationary operand between matmuls), which removes the dominant
  ~130ns/matmul weight-reload tax of a chunk-major loop.  b1 rides on
  the relu activation's per-partition bias port (fp32, not fp8).

  FC2 packs the four chunks' [10, 512] logit tiles into one PSUM bank
  at partition strips {0,32,64,96} via tile_position column groups;
  consecutive strip matmuls stream different chunks through different
  XBUSes and overlap in the array.  The log-softmax partition
  reductions use the ones-matmul trick on diagonal (row,col)=(32c,32c)
  tiles, and exp/ln/bias-add/sub each run as ONE wide instruction over
  all strips instead of per-chunk.  Relus alternate between ScalarE
  and VectorE so neither engine sits on the critical path.

    H'.T[hid, b] = sum_t W18[t].T @ X8[t]                   (fp32 acc)
    L.T[strip c][10, b] = sum_k (W4/64)[k].T @ H'.T[k, b]
    out.T = (L.T + b4) - ln(ones.T @ exp(L.T + b4))
"""

import numpy as np
import ml_dtypes

import concourse.bass as bass
import concourse.bacc as bacc
import concourse.mybir as mybir
import concourse.tile as tile
from concourse.bass_utils import run_bass_kernel_spmd

BF16 = mybir.dt.bfloat16
FP8 = mybir.dt.float8e4
F32 = mybir.dt.float32
AF = mybir.ActivationFunctionType
DR = mybir.MatmulPerfMode.DoubleRow
ALU = mybir.AluOpType

B, H, W = 32768, 28, 28
PIX = H * W            # 784
HID = 800
NCLS = 10
NCORES = 8
BPC = B // NCORES      # 4096 images per core
NB = 512               # batch chunk = one PSUM bank of fp32
NCH = BPC // NB        # 8 chunks per core
NT8 = 3                # full fp8 DoubleRow tiles (256 pixels each = 768)
P8 = NT8 * 256         # pixels covered by the full fp8 tiles
KT3 = 8                # tail tile partitions: 16 virtual rows = 16 px
WS = 64.0              # host-side scale on w1/b1 (folded out via w4)
HT = (HID + 127) // 128          # 7 contraction tiles for fc2
M_TILES = [(m * 128, min(128, HID - m * 128)) for m in range(HT)]
GROUPS = [(0, 4), (4, 4)]        # (first chunk, chunks per group)

_built = None  # nc cache — BIR build is pure host work


def _dedup_ldweights(nc):
    """Remove Ldweights whose weight AP/mode matches the immediately
    preceding Ldweights on the tensor queue: the PE array keeps its
    stationary operand between matmuls, so repeated loads are pure
    overhead.  Any waits/updates on a removed load migrate to the next
    tensor-engine instruction (its matmul)."""
    removed = 0
    for blk in nc.main_func.blocks:
        last_sig = None
        to_remove = []
        for ins in blk.instructions:
            op = ins.opcode
            if op == 'Ldweights':
                sig = (str(ins.ins[0]), str(ins.perf_mode),
                       str(ins.tile_position), str(ins.is_transpose))
                if sig == last_sig:
                    to_remove.append(ins)
                    continue
                last_sig = sig
        for ins in to_remove:
            arr = blk.instructions
            idx = next(i for i in range(len(arr)) if arr[i].name == ins.name)
            # next tensor-engine instruction inherits the sync info
            nxt = None
            for j in range(idx + 1, len(arr)):
                if arr[j].engine == ins.engine:
                    nxt = arr[j]
                    break
            assert nxt is not None
            si = ins.sync_info
            if si is not None and (len(si.on_wait) or len(si.on_update)):
                nsi = nxt.sync_info
                if nsi is None:
                    nxt.sync_info = mybir.SyncInfo(
                        on_wait=list(si.on_wait), on_update=list(si.on_update))
                else:
                    nsi.on_wait = list(nsi.on_wait) + list(si.on_wait)
                    nsi.on_update = list(nsi.on_update) + list(si.on_update)
            del arr[idx]
            removed += 1
    return removed


def _build_device_program():
    nc = bacc.Bacc(
        "TRN2", target_bir_lowering=False, debug=False, num_devices=NCORES
    )
    # x8: pixel k = 256 t + 2 p + i, image n = 512 c + j.
    x8_d = nc.dram_tensor("x8", [NT8, 128, NCH, 2, NB], FP8, kind="ExternalInput")
    # tail: virtual row v = 2 p + i -> pixel 768 + v.
    x3_d = nc.dram_tensor("x3", [KT3, NCH, 2, NB], FP8, kind="ExternalInput")
    w18_d = nc.dram_tensor("w18", [128, NT8, 2, HID], FP8, kind="ExternalInput")
    wt3_d = nc.dram_tensor("wt3", [KT3, 2, HID], FP8, kind="ExternalInput")
    w4_d = nc.dram_tensor("w4", [128, HT, NCLS], BF16, kind="ExternalInput")
    b1_d = nc.dram_tensor("b1", [128, HT], F32, kind="ExternalInput")
    b4r_d = nc.dram_tensor("b4r", [128, 1], F32, kind="ExternalInput")
    # full-height dump, one DMA per group; host extracts partition strips
    out_d = nc.dram_tensor("outT", [128, len(GROUPS), NB], F32, kind="ExternalOutput")

    # One ACT-function table containing relu/exp/ln (avoids LUT reloads).
    from concourse.hw_specs import get_activation_tables
    needed = {AF.Relu, AF.Exp, AF.Ln, AF.Identity, AF.Copy}
    table_id = None
    for i, (name, funcs) in enumerate(get_activation_tables(nc.m.arch).items()):
        if needed <= funcs:
            table_id = i
            break

    with tile.TileContext(nc) as tc:
        with (
            tc.tile_pool(name="weights", bufs=1) as wpool,
            tc.tile_pool(name="xin", bufs=2) as xpool,
            tc.tile_pool(name="hmid", bufs=40) as hpool,
            tc.tile_pool(name="smax", bufs=2) as spool,
            tc.tile_pool(name="psum_h", bufs=6, space="PSUM") as php,
            tc.tile_pool(name="psum_l", bufs=1, space="PSUM") as plp,
            tc.tile_pool(name="psum_t", bufs=1, space="PSUM") as smp,
        ):
            # ---- startup DMAs: first-needed first, on separate queues ----
            w18_sb = wpool.tile([128, NT8, 2, HID], FP8)
            nc.scalar.dma_start(w18_sb[:, 0:1, :, :], w18_d[:, 0:1, :, :])

            xg = {}   # (group, t) -> [128, G, 2, NB]
            x3g = {}  # group -> [KT3, G, 2, NB]

            def load_group(gi):
                c0, G = GROUPS[gi]
                # queue order: t0 (needed first), x3 (tiny, needed at t=3),
                # then t1, t2
                for t in range(NT8):
                    xt = xpool.tile([128, G, 2, NB], FP8, tag=f"x{t}",
                                    name=f"x{t}g{gi}")
                    nc.sync.dma_start(xt[:, :, :, :], x8_d[t, :, c0:c0 + G, :, :])
                    xg[(gi, t)] = xt
                    if t == 0:
                        x3t = xpool.tile([KT3, G, 2, NB], FP8, tag="x3",
                                         name=f"x3t{gi}")
                        nc.sync.dma_start(x3t[:, :, :, :], x3_d[:, c0:c0 + G, :, :])
                        x3g[gi] = x3t

            load_group(0)

            nc.scalar.dma_start(w18_sb[:, 1:NT8, :, :], w18_d[:, 1:NT8, :, :])
            wt3_sb = wpool.tile([KT3, 2, HID], FP8)
            nc.scalar.dma_start(wt3_sb[:, :, :], wt3_d[:, :, :])

            if table_id is not None:
                nc.scalar.add_instruction(
                    mybir.InstLoadActFuncSet(
                        name=nc.get_next_instruction_name(),
                        act_func_set_id=table_id,
                        ins=[],
                        outs=[],
                    )
                )

            w4_sb = wpool.tile([128, HT, NCLS], BF16)
            nc.gpsimd.dma_start(w4_sb[:, :, :], w4_d[:, :, :])
            b1_sb = wpool.tile([128, HT], F32)
            nc.gpsimd.dma_start(b1_sb[:, :], b1_d[:, :])
            b4r_sb = wpool.tile([128, 1], F32)
            nc.gpsimd.dma_start(b4r_sb[:, :], b4r_d[:, :])
            ones_sb = wpool.tile([128, 16], BF16)
            nc.gpsimd.memset(ones_sb[:, :], 1.0)

            # Software-pipelined softmax: group g's partition reductions
            # (PE) are emitted inside group g+1's FC1 m-loop so the PE
            # never stalls on ScalarE's exp/ln.
            stage1 = None  # (exp_sb, logit_sb, c0, G)
            stage2 = None  # (lse_sb, logit_sb, c0, G)

            def softmax_stage1():
                nonlocal stage1, stage2
                if stage1 is None:
                    return
                exp_sb, logit_sb, gi1, G = stage1
                stage1 = None
                ps_t = smp.tile([128, NB], F32, tag="sm")
                for ci in range(G):
                    s = 32 * ci
                    nc.tensor.matmul(
                        ps_t[s:s + 1, :],
                        ones_sb[s:s + NCLS, 0:1],
                        exp_sb[s:s + NCLS, :],
                        tile_position=(s, s),
                    )
                lse_sb = spool.tile([128, NB], BF16, tag="lse")
                top1 = 32 * (G - 1) + 1
                nc.scalar.activation(lse_sb[:top1, :], ps_t[:top1, :], AF.Ln)
                stage2 = (lse_sb, logit_sb, gi1, G)

            def softmax_stage2():
                nonlocal stage2
                if stage2 is None:
                    return
                lse_sb, logit_sb, gi2, G = stage2
                stage2 = None
                top = 32 * (G - 1) + NCLS
                pb_t = smp.tile([128, NB], F32, tag="sm")
                for ci in range(G):
                    s = 32 * ci
                    nc.tensor.matmul(
                        pb_t[s:s + NCLS, :],
                        ones_sb[s:s + 1, 0:NCLS],
                        lse_sb[s:s + 1, :],
                        tile_position=(s, s),
                    )
                out_sb = spool.tile([128, NB], F32, tag="outc")
                nc.vector.tensor_sub(out_sb[:top, :], logit_sb[:top, :], pb_t[:top, :])
                nc.gpsimd.dma_start(out_d[:, gi2, :], out_sb[:, :])

            for gi, (c0, G) in enumerate(GROUPS):
                top = 32 * (G - 1) + NCLS
                hts = {}  # (ci, m) -> relu'd fc1 tile
                pl_t = None

                def fc2_block(k):
                    kk = min(128, HID - k * 128)
                    for ci in range(G):
                        s = 32 * ci
                        nc.tensor.matmul(
                            pl_t[s:s + NCLS, :],
                            w4_sb[:kk, k, :],
                            hts[(ci, k)][:kk, :],
                            start=(k == 0),
                            stop=(k == HT - 1),
                            tile_position=(0, s),
                        )

                for m, (m0, mm) in enumerate(M_TILES):
                    ph_t = [
                        php.tile([128, NB], F32, tag="ph", name=f"ph{gi}_{m}_{ci}")
                        for ci in range(G)
                    ]
                    for t in range(NT8 + 1):
                        for ci in range(G):
                            if t < NT8:
                                lhsT = w18_sb[:, t, :, m0:m0 + mm]
                                rhs = xg[(gi, t)][:, ci, :, :]
                            else:
                                lhsT = wt3_sb[:, :, m0:m0 + mm]
                                rhs = x3g[gi][:, ci, :, :]
                            nc.tensor.matmul(
                                ph_t[ci][:mm, :],
                                lhsT,
                                rhs,
                                start=(t == 0),
                                stop=(t == NT8),
                                perf_mode=DR,
                            )
                    for ci in range(G):
                        ht_t = hpool.tile([128, NB], BF16, tag="ht")
                        if (m + ci) % 2 == 0:
                            nc.scalar.activation(
                                ht_t[:mm, :], ph_t[ci][:mm, :], AF.Relu,
                                bias=b1_sb[:mm, m:m + 1],
                            )
                        else:
                            nc.vector.tensor_scalar(
                                ht_t[:mm, :], ph_t[ci][:mm, :],
                                b1_sb[:mm, m:m + 1], 0.0, ALU.add, ALU.max,
                            )
                        hts[(ci, m)] = ht_t
                    if m == 0:
                        pl_t = plp.tile([128, NB], F32, tag="pl")
                        if gi + 1 < len(GROUPS):
                            load_group(gi + 1)  # prefetch next group's x
                    if m >= 1:
                        fc2_block(m - 1)  # this group's fc2, k = m-1
                    if m == 1:
                        softmax_stage1()  # previous group
                    elif m == 3:
                        softmax_stage2()  # previous group

                fc2_block(HT - 1)
                exp_sb = spool.tile([128, NB], BF16, tag="exp")
                nc.scalar.activation(
                    exp_sb[:top, :], pl_t[:top, :], AF.Exp, bias=b4r_sb[:top, 0:1]
                )
                logit_sb = spool.tile([128, NB], F32, tag="logit")
                nc.vector.tensor_scalar_add(
                    logit_sb[:top, :], pl_t[:top, :], b4r_sb[:top, 0:1]
                )
                stage1 = (exp_sb, logit_sb, gi, G)

            softmax_stage1()
            softmax_stage2()

    n = _dedup_ldweights(nc)
    assert n > 0, "ldweights dedup pass matched nothing"
    nc.finalize()
    return nc


def _optical_operator(tmask_re, tmask_im):
    """A such that xr_flat = A @ x_flat for the masked centered FFT front."""
    tmask = tmask_re.astype(np.complex64) + 1j * tmask_im.astype(np.complex64)
    tmask = tmask.reshape(H, W)
    ax = (-2, -1)
    eye = np.eye(PIX, dtype=np.complex64).reshape(PIX, H, W)
    f = np.fft.fftshift(np.fft.fft2(np.fft.ifftshift(eye, axes=ax), axes=ax), axes=ax)
    f *= tmask[None, :, :]
    xr = np.fft.fftshift(np.fft.ifft2(np.fft.ifftshift(f, axes=ax), axes=ax), axes=ax)
    return np.real(xr).reshape(PIX, PIX).T.astype(np.float64)


def kernel(x, tmask_re, tmask_im, w1, b1, w4, b4):
    global _built
    x = np.asarray(x)
    w1 = np.asarray(w1, dtype=np.float32)
    b1 = np.asarray(b1, dtype=np.float32)
    w4 = np.asarray(w4, dtype=np.float32)
    b4 = np.asarray(b4, dtype=np.float32)
    tre = np.asarray(tmask_re, dtype=np.float32)
    tim = np.asarray(tmask_im, dtype=np.float32)

    # Fold the optical front into w1.  Identity mask -> A == I exactly.
    if np.all(tre == 1.0) and np.all(tim == 0.0):
        w1e = w1.astype(np.float64)
    else:
        w1e = w1.astype(np.float64) @ _optical_operator(tre, tim)

    bf16 = ml_dtypes.bfloat16
    fp8 = ml_dtypes.float8_e4m3fn

    def q8(a):
        return np.clip(a, -240, 240).astype(fp8)

    # w1 scaled by WS so fp8 encodings sit in e4m3's normal range; the
    # matching 1/WS rides on w4 (relu commutes with positive scaling).
    w1s = (w1e * WS).astype(np.float32)
    # full fp8 tiles: [hid, 768] -> [128 p, 3 t, 2 i, hid], k = 256t+2p+i
    w18 = np.ascontiguousarray(
        q8(w1s[:, :P8]).reshape(HID, NT8, 128, 2).transpose(2, 1, 3, 0)
    )
    # tail tile: rows v = 2p+i -> pixel 768+v (v < 16)
    wt3 = np.ascontiguousarray(
        q8(w1s[:, P8:]).reshape(HID, KT3, 2).transpose(1, 2, 0)
    )
    w4t = np.zeros((HT * 128, NCLS), dtype=np.float32)
    w4t[:HID, :] = (w4 / WS).T
    w4c = np.ascontiguousarray(
        w4t.reshape(HT, 128, NCLS).transpose(1, 0, 2).astype(bf16)
    )
    b1f = np.zeros(HT * 128, dtype=np.float32)
    b1f[:HID] = b1 * WS
    b1c = np.ascontiguousarray(b1f.reshape(HT, 128).T)
    b4r = np.zeros((128, 1), dtype=np.float32)
    for c in range(4):
        b4r[32 * c:32 * c + NCLS, 0] = b4

    # x: [B, 784] fp8 packed as [3 t, 128 p, 8 c, 2 i, 512 j] per core,
    # plus the tail tile [8 p, 8 c, 2 i, 512 j].
    xf = x.reshape(B, PIX)
    x8a = q8(xf[:, :P8])       # [B, 768]
    x3a = q8(xf[:, P8:])       # [B, 16]

    if _built is None:
        _built = _build_device_program()
    nc = _built

    in_maps = []
    for c in range(NCORES):
        sl = slice(c * BPC, (c + 1) * BPC)
        # [BPC, 768] -> [8, 512, 3, 128, 2] -> [3, 128, 8, 2, 512]
        x8c = np.ascontiguousarray(
            x8a[sl].reshape(NCH, NB, NT8, 128, 2).transpose(2, 3, 0, 4, 1)
        )
        # [BPC, 16] -> [8, 512, 8, 2] -> [8, 8, 2, 512]
        x3c = np.ascontiguousarray(
            x3a[sl].reshape(NCH, NB, KT3, 2).transpose(2, 0, 3, 1)
        )
        in_maps.append({
            "x8": x8c,
            "x3": x3c,
            "w18": w18,
            "wt3": wt3,
            "w4": w4c,
            "b1": b1c,
            "b4r": b4r,
        })
    res = run_bass_kernel_spmd(nc, in_maps, core_ids=list(range(NCORES)))

    out = np.empty((B, NCLS), dtype=np.float32)
    for c in range(NCORES):
        ob = res.results[c]["outT"]  # [128, n_groups, NB]
        for gi, (c0, G) in enumerate(GROUPS):
            for ci in range(G):
                ch = c0 + ci
                sl = slice(c * BPC + ch * NB, c * BPC + (ch + 1) * NB)
                out[sl, :] = ob[32 * ci:32 * ci + NCLS, gi, :].T
    return out
